# revision 3
# baseline (speedup 1.0000x reference)
"""Trainium2 Bass kernel for nn_DiffusionConv (two-direction GCN conv + relu).

out = relu(gcn(x, W_fwd; src->dst) + gcn(x, W_bwd; dst->src))

Algorithm (validated against the jax reference):
  gcn(x, W; edges) = D^-1/2 (A + I) D^-1/2 x W  with D = indegree+1.
  The weight GEMM commutes with aggregation: the device aggregates scaled
  features u = dinv * x over in-edges of each 128-row destination tile,
  applies W, scales by destination dinv, adds bias, relu.

Device mapping (one SPMD program on 8 cores, no SWDGE gathers):
  - The edge gather is done on the HOST: preprocess() writes, per core and
    per conv, a partition-major message table tbl[lane, chunk*128+f] in
    DRAM holding u[src] for every edge lane (plus one self-loop chunk per
    tile). The device streams it with large contiguous HWDGE DMAs.
  - Scatter into destination slots runs on the PE: for each 128-edge chunk,
    agg_T += G_c.T @ M_c where M_c[lane, r] = (dst_slot[lane] == r).
    M is built on the DVE in the [lane, r, chunk] layout (packed last dim,
    broadcast middle) which qualifies for the DVE 2x perf mode; the PE
    reads rhs slices M[:, :, c] with a free-dim stride.
  - Self-loop chunks use a constant identity rhs (no M build).
  - Per tile: bf16 GEMM with W, then dinv scaling (Act), combine (DVE),
    bias + relu (Pool), batched group store (Act HWDGE).
  - Tiles are degree-sorted and assigned to (core, slot) so that all cores
    share one chunk schedule with minimal padding.
"""

import sys

if "/opt/trn_rl_repo" not in sys.path:
    sys.path.insert(0, "/opt/trn_rl_repo")

import numpy as np
import ml_dtypes

P = 128
N_CORES = 8
BF16 = np.dtype(ml_dtypes.bfloat16)


class Schedule:
    pass


def preprocess(x, edge_index, W_fwd, b_fwd, W_bwd, b_bwd,
               group_tiles=7, n_cores=N_CORES):
    N, D = x.shape
    assert D == P
    src = edge_index[0].astype(np.int64)
    dst = edge_index[1].astype(np.int64)

    deg_f = np.bincount(dst, minlength=N) + 1.0
    deg_b = np.bincount(src, minlength=N) + 1.0
    dinv_f = (1.0 / np.sqrt(deg_f)).astype(np.float32)
    dinv_b = (1.0 / np.sqrt(deg_b)).astype(np.float32)

    u_f = (dinv_f[:, None] * x).astype(np.float16)
    u_b = (dinv_b[:, None] * x).astype(np.float16)

    n_tiles = -(-N // P)
    n_tiles = -(-n_tiles // n_cores) * n_cores
    T = n_tiles // n_cores

    # node -> tile: plain blocks; tile -> (core, slot) by degree-sorted
    # ranking so the 8 tiles sharing a slot have near-equal chunk counts.
    node_tile = np.arange(N) // P
    node_pos = np.arange(N) % P

    E_f = np.bincount(node_tile[dst], minlength=n_tiles)
    E_b = np.bincount(node_tile[src], minlength=n_tiles)
    order = np.argsort(-(E_f + E_b), kind="stable")
    tile_slot = np.empty(n_tiles, dtype=np.int64)
    tile_core = np.empty(n_tiles, dtype=np.int64)
    rank = np.arange(n_tiles)
    tile_slot[order] = rank // n_cores
    core_seq = rank % n_cores
    odd = (rank // n_cores) % 2 == 1
    core_seq[odd] = n_cores - 1 - core_seq[odd]
    tile_core[order] = core_seq

    # per-slot chunk counts: max over the 8 cores sharing the slot
    def slot_K(E):
        per = np.zeros(T, dtype=np.int64)
        np.maximum.at(per, tile_slot, E)
        return -(-per // P)

    K_f = slot_K(E_f)
    K_b = slot_K(E_b)
    kmax = max(int(K_f.max()), int(K_b.max()), 2)

    # chunk/column layout per conv: per slot t -> [self | K[t] edge chunks]
    nch_f = 1 + K_f
    nch_b = 1 + K_b
    cb_f = np.concatenate([[0], np.cumsum(nch_f)])   # chunk base per slot
    cb_b = np.concatenate([[0], np.cumsum(nch_b)])
    NCHF, NCHB = int(cb_f[-1]), int(cb_b[-1])
    # pos column base per slot (edge chunks only), f then b merged
    pb_f = np.concatenate([[0], np.cumsum(K_f)])
    pb_b = np.concatenate([[0], np.cumsum(K_b)]) + pb_f[-1]
    PC = int(pb_b[-1])

    groups = [list(range(g, min(g + group_tiles, T)))
              for g in range(0, T, group_tiles)]

    # core/slot -> global tile id
    cs_tile = np.full((n_cores, T), -1, dtype=np.int64)
    cs_tile[tile_core, tile_slot] = np.arange(n_tiles)

    iota3 = np.ascontiguousarray(
        np.broadcast_to(np.arange(P, dtype=np.float16)[None, :, None],
                        (P, P, kmax)))
    ident = np.eye(P, dtype=np.float16)
    wf = np.ascontiguousarray(W_fwd.astype(BF16))
    wb = np.ascontiguousarray(W_bwd.astype(BF16))
    bias_mat = np.ascontiguousarray(np.broadcast_to(
        (b_fwd + b_bwd).astype(np.float32)[None, :], (P, P)))

    in_maps = []
    for core in range(n_cores):
        tiles = cs_tile[core]                     # [T] global tile ids
        nodes = tiles[:, None] * P + np.arange(P)[None, :]  # [T, P]
        valid = nodes < N
        nidx = np.where(valid, nodes, 0)

        def build(keys, gidx, u, K, cb, pb, NCH):
            # keys = destination node per edge, gidx = source node
            m = tile_core[node_tile[keys]] == core
            kk, gg = keys[m], gidx[m]
            t_loc = tile_slot[node_tile[kk]]
            o = np.argsort(t_loc, kind="stable")
            t_loc, kk, gg = t_loc[o], kk[o], gg[o]
            starts = np.searchsorted(t_loc, np.arange(T))
            cnt = np.arange(len(kk)) - starts[t_loc]
            chunk = cb[t_loc] + 1 + cnt // P
            lane = cnt % P
            tbl = np.zeros((P, NCH, P), dtype=np.float16)
            tbl[lane, chunk, :] = u[gg]
            # self chunks: permuted u rows of the tile's own nodes
            selfrows = np.where(valid[:, :, None], u[nidx], 0)  # [T, P, P]
            tbl[:, cb[:-1], :] = selfrows.transpose(1, 0, 2)
            posm = np.zeros((P, int(K.sum())), dtype=np.float16)
            posm[lane, pb[t_loc] - pb[0] + cnt // P] = (kk % P)
            return tbl.reshape(P, NCH * P), posm

        tbl_f, pos_f = build(dst, src, u_f, K_f, cb_f, pb_f, NCHF)
        tbl_b, pos_b = build(src, dst, u_b, K_b, cb_b, pb_b, NCHB)
        pos = np.concatenate([pos_f, pos_b], axis=1)
        assert pos.shape[1] == PC

        dvf = np.zeros((P, T), dtype=np.float32)
        dvb = np.zeros((P, T), dtype=np.float32)
        dvf.T[valid] = dinv_f[nidx[valid]]
        dvb.T[valid] = dinv_b[nidx[valid]]

        in_maps.append({
            "tblf": tbl_f, "tblb": tbl_b, "pos": np.ascontiguousarray(pos),
            "dinvf": np.ascontiguousarray(dvf),
            "dinvb": np.ascontiguousarray(dvb),
            "wf": wf, "wb": wb, "bias": bias_mat,
            "iota": iota3, "ident": ident,
        })

    sch = Schedule()
    sch.T, sch.kmax, sch.groups = T, kmax, groups
    sch.K_f, sch.K_b = K_f, K_b
    sch.cb_f, sch.cb_b = cb_f, cb_b
    sch.pb_f, sch.pb_b = pb_f, pb_b
    sch.NCHF, sch.NCHB, sch.PC = NCHF, NCHB, PC
    sch.in_maps = in_maps
    sch.cs_tile = cs_tile
    sch.N, sch.n_cores = N, n_cores
    return sch


# ---------------------------------------------------------------------------
# device program
# ---------------------------------------------------------------------------

def build_program(sch, debug=False, dup=1):
    from contextlib import ExitStack
    import concourse.mybir as mybir
    import concourse.tile as tile
    from concourse import bacc

    f16 = mybir.dt.float16
    bf16 = mybir.dt.bfloat16
    f32 = mybir.dt.float32
    T, kmax = sch.T, sch.kmax
    K_f, K_b = sch.K_f, sch.K_b
    cb_f, cb_b = sch.cb_f, sch.cb_b
    pb_f, pb_b = sch.pb_f, sch.pb_b

    nc = bacc.Bacc("TRN2", target_bir_lowering=False, debug=debug,
                   num_devices=sch.n_cores, num_swdge_queues=1)

    tblf_d = nc.dram_tensor("tblf", [P, sch.NCHF * P], f16,
                            kind="ExternalInput").ap()
    tblb_d = nc.dram_tensor("tblb", [P, sch.NCHB * P], f16,
                            kind="ExternalInput").ap()
    pos_d = nc.dram_tensor("pos", [P, sch.PC], f16,
                           kind="ExternalInput").ap()
    dinvf_d = nc.dram_tensor("dinvf", [P, T], f32, kind="ExternalInput").ap()
    dinvb_d = nc.dram_tensor("dinvb", [P, T], f32, kind="ExternalInput").ap()
    wf_d = nc.dram_tensor("wf", [P, P], bf16, kind="ExternalInput").ap()
    wb_d = nc.dram_tensor("wb", [P, P], bf16, kind="ExternalInput").ap()
    bias_d = nc.dram_tensor("bias", [P, P], f32, kind="ExternalInput").ap()
    iota_d = nc.dram_tensor("iota", [P, P, kmax], f16,
                            kind="ExternalInput").ap()
    ident_d = nc.dram_tensor("ident", [P, P], f16,
                             kind="ExternalInput").ap()
    out_d = nc.dram_tensor("out", [P, T * P], f32, kind="ExternalOutput").ap()

    with tile.TileContext(nc) as tc, ExitStack() as ctx:
        const = ctx.enter_context(tc.tile_pool(name="const", bufs=1))

        def load_const(shape, dt, dram_ap, tag):
            t = const.tile(shape, dtype=dt, tag=tag)
            nc.sync.dma_start(out=t[:], in_=dram_ap)
            return t

        pos_sb = load_const([P, sch.PC], f16, pos_d, "pos")
        dinvf_sb = load_const([P, T], f32, dinvf_d, "dinvf")
        dinvb_sb = load_const([P, T], f32, dinvb_d, "dinvb")
        wf_sb = load_const([P, P], bf16, wf_d, "wf")
        wb_sb = load_const([P, P], bf16, wb_d, "wb")
        bias_sb = load_const([P, P], f32, bias_d, "bias")
        iota_sb = load_const([P, P, kmax], f16, iota_d, "iota")
        ident_sb = load_const([P, P], f16, ident_d, "ident")

        gpf = ctx.enter_context(tc.tile_pool(name="gf", bufs=2))
        gpb = ctx.enter_context(tc.tile_pool(name="gb", bufs=2))
        mpool = ctx.enter_context(tc.tile_pool(name="m", bufs=4))
        aggp = ctx.enter_context(tc.tile_pool(name="aggp", bufs=2,
                                              space="PSUM"))
        outp = ctx.enter_context(tc.tile_pool(name="outp", bufs=2,
                                              space="PSUM"))
        spool = ctx.enter_context(tc.tile_pool(name="sp", bufs=4))
        s1pool = ctx.enter_context(tc.tile_pool(name="s1p", bufs=3))
        s2pool = ctx.enter_context(tc.tile_pool(name="s2p", bufs=3))
        s3pool = ctx.enter_context(tc.tile_pool(name="s3p", bufs=3))
        ogrp = ctx.enter_context(tc.tile_pool(name="og", bufs=2))

        for _dup in range(dup):
            for g in sch.groups:
                g0, g1 = g[0], g[-1] + 1
                ncf = int(cb_f[g1] - cb_f[g0])
                ncb = int(cb_b[g1] - cb_b[g0])
                gtf = gpf.tile([P, ncf * P], dtype=f16, tag="gtf")
                nc.sync.dma_start(
                    out=gtf[:],
                    in_=tblf_d[:, int(cb_f[g0]) * P:int(cb_f[g1]) * P])
                gtb = gpb.tile([P, ncb * P], dtype=f16, tag="gtb")
                nc.sync.dma_start(
                    out=gtb[:],
                    in_=tblb_d[:, int(cb_b[g0]) * P:int(cb_b[g1]) * P])

                og_t = ogrp.tile([P, len(g), P], dtype=f32, tag="og")

                for ti, t in enumerate(g):
                    def conv(gt, cb, pb, K, w_sb, tag):
                        kst = int(K[t])
                        base = int(cb[t] - cb[g0]) * P
                        agg = aggp.tile([P, P], dtype=f32, tag=f"agg{tag}")
                        nc.tensor.matmul(out=agg[:],
                                         lhsT=gt[:, base:base + P],
                                         rhs=ident_sb[:],
                                         start=True, stop=(kst == 0))
                        if kst > 0:
                            mt = mpool.tile([P, P, kst], dtype=f16,
                                            tag=f"m{tag}")
                            p0 = int(pb[t])
                            nc.vector.tensor_tensor(
                                out=mt[:],
                                in0=pos_sb[:, None, p0:p0 + kst]
                                .to_broadcast([P, P, kst]),
                                in1=iota_sb[:, :, 0:kst],
                                op=mybir.AluOpType.is_equal)
                            for c in range(kst):
                                nc.tensor.matmul(
                                    out=agg[:],
                                    lhsT=gt[:, base + (1 + c) * P:
                                            base + (2 + c) * P],
                                    rhs=mt[:, :, c],
                                    start=False, stop=(c == kst - 1))
                        a_sb = spool.tile([P, P], dtype=bf16, tag=f"a{tag}")
                        nc.scalar.copy(out=a_sb[:], in_=agg[:])
                        o_ps = outp.tile([P, P], dtype=f32, tag=f"o{tag}")
                        nc.tensor.matmul(out=o_ps[:], lhsT=a_sb[:],
                                         rhs=w_sb[:], start=True, stop=True)
                        return o_ps

                    outf = conv(gtf, cb_f, pb_f, K_f, wf_sb, "f")
                    outb = conv(gtb, cb_b, pb_b, K_b, wb_sb, "b")

                    s1 = s1pool.tile([P, P], dtype=f32, tag="s1")
                    nc.scalar.mul(out=s1[:], in_=outf[:],
                                  mul=dinvf_sb[:, t:t + 1])
                    s2 = s2pool.tile([P, P], dtype=f32, tag="s2")
                    nc.vector.scalar_tensor_tensor(
                        out=s2[:], in0=outb[:],
                        scalar=dinvb_sb[:, t:t + 1], in1=s1[:],
                        op0=mybir.AluOpType.mult,
                        op1=mybir.AluOpType.add)
                    s3 = s3pool.tile([P, P], dtype=f32, tag="s3")
                    nc.gpsimd.tensor_tensor(out=s3[:], in0=s2[:],
                                            in1=bias_sb[:],
                                            op=mybir.AluOpType.add)
                    nc.gpsimd.tensor_scalar(
                        out=og_t[:, ti, :], in0=s3[:],
                        scalar1=0.0, scalar2=None,
                        op0=mybir.AluOpType.max)

                nc.scalar.dma_start(out=out_d[:, g0 * P:g1 * P],
                                    in_=og_t[:])

    nc.compile()
    return nc


# ---------------------------------------------------------------------------
# entry point
# ---------------------------------------------------------------------------

_CACHE = {}


def run_sch(sch, trace=False, **kw):
    from concourse.bass_utils import run_bass_kernel_spmd
    key = ("prog", sch.T, sch.kmax, sch.NCHF, sch.NCHB, sch.PC)
    if key not in _CACHE:
        _CACHE.clear()
        _CACHE[key] = build_program(sch)
    nc = _CACHE[key]
    return run_bass_kernel_spmd(
        nc, sch.in_maps, core_ids=list(range(sch.n_cores)), trace=trace, **kw)


def assemble(sch, results):
    out = np.zeros((sch.N, P), dtype=np.float32)
    for core in range(sch.n_cores):
        o = results[core]["out"].reshape(P, sch.T, P).transpose(1, 0, 2)
        tiles = sch.cs_tile[core]
        nodes = tiles[:, None] * P + np.arange(P)[None, :]
        valid = nodes < sch.N
        out[nodes[valid]] = o[valid]
    return out


def kernel(x, edge_index, W_fwd, b_fwd, W_bwd, b_bwd):
    x = np.asarray(x, dtype=np.float32)
    edge_index = np.asarray(edge_index, dtype=np.int32)
    sch = preprocess(
        x, edge_index,
        np.asarray(W_fwd, np.float32), np.asarray(b_fwd, np.float32),
        np.asarray(W_bwd, np.float32), np.asarray(b_bwd, np.float32))
    res = run_sch(sch)
    return assemble(sch, res.results)


# revision 8
# speedup vs baseline: 3.4066x; 3.4066x over previous
"""Trainium2 Bass kernel for nn_DiffusionConv (two-direction GCN conv + relu).

out = relu(gcn(x, W_fwd; src->dst) + gcn(x, W_bwd; dst->src))

Algorithm (validated against the jax reference):
  gcn(x, W; edges) = D^-1/2 (A + I) D^-1/2 x W  with D = indegree+1.
  The weight GEMM commutes with aggregation: the device aggregates scaled
  features u = dinv * x over in-edges of each 128-row destination tile,
  applies W, scales by destination dinv, adds bias, relu.

Device mapping (one SPMD program on 8 cores, no SWDGE gathers):
  - The edge gather is done on the HOST: preprocess() writes, per core and
    per conv, a partition-major message table tbl[lane, chunk*128+f] in
    DRAM holding u[src] for every edge lane (plus one self-loop chunk per
    tile). The device streams it with large contiguous HWDGE DMAs.
  - Scatter into destination slots runs on the PE: for each 128-edge chunk,
    agg_T += G_c.T @ M_c where M_c[lane, r] = (dst_slot[lane] == r).
    M is built on the DVE in the [lane, r, chunk] layout (packed last dim,
    broadcast middle) which qualifies for the DVE 2x perf mode; the PE
    reads rhs slices M[:, :, c] with a free-dim stride.
  - Self-loop chunks use a constant identity rhs (no M build).
  - Per tile: bf16 GEMM with W, then dinv scaling (Act), combine (DVE),
    bias + relu (Pool), batched group store (Act HWDGE).
  - Tiles are degree-sorted and assigned to (core, slot) so that all cores
    share one chunk schedule with minimal padding.
"""

import sys

if "/opt/trn_rl_repo" not in sys.path:
    sys.path.insert(0, "/opt/trn_rl_repo")

import numpy as np
import ml_dtypes

P = 128
N_CORES = 8
BF16 = np.dtype(ml_dtypes.bfloat16)


class Schedule:
    pass


def preprocess(x, edge_index, W_fwd, b_fwd, W_bwd, b_bwd,
               group_tiles=5, n_cores=N_CORES):
    N, D = x.shape
    assert D == P
    src = edge_index[0].astype(np.int64)
    dst = edge_index[1].astype(np.int64)

    deg_f = np.bincount(dst, minlength=N) + 1.0
    deg_b = np.bincount(src, minlength=N) + 1.0
    dinv_f = (1.0 / np.sqrt(deg_f)).astype(np.float32)
    dinv_b = (1.0 / np.sqrt(deg_b)).astype(np.float32)

    u_f = (dinv_f[:, None] * x).astype(np.float16)
    u_b = (dinv_b[:, None] * x).astype(np.float16)

    n_tiles = -(-N // P)
    n_tiles = -(-n_tiles // n_cores) * n_cores
    T = n_tiles // n_cores

    # node -> tile: plain blocks; tile -> (core, slot) by degree-sorted
    # ranking so the 8 tiles sharing a slot have near-equal chunk counts.
    node_tile = np.arange(N) // P
    node_pos = np.arange(N) % P

    E_f = np.bincount(node_tile[dst], minlength=n_tiles)
    E_b = np.bincount(node_tile[src], minlength=n_tiles)
    order = np.argsort(-(E_f + E_b), kind="stable")
    tile_slot = np.empty(n_tiles, dtype=np.int64)
    tile_core = np.empty(n_tiles, dtype=np.int64)
    rank = np.arange(n_tiles)
    tile_slot[order] = rank // n_cores
    core_seq = rank % n_cores
    odd = (rank // n_cores) % 2 == 1
    core_seq[odd] = n_cores - 1 - core_seq[odd]
    tile_core[order] = core_seq

    # per-slot chunk counts: max over the 8 cores sharing the slot
    def slot_K(E):
        per = np.zeros(T, dtype=np.int64)
        np.maximum.at(per, tile_slot, E)
        return -(-per // P)

    K_f = slot_K(E_f)
    K_b = slot_K(E_b)
    kmax = max(int(K_f.max()), int(K_b.max()), 2)

    # chunk/column layout per conv: per slot t -> [self | K[t] edge chunks]
    nch_f = 1 + K_f
    nch_b = 1 + K_b
    cb_f = np.concatenate([[0], np.cumsum(nch_f)])   # chunk base per slot
    cb_b = np.concatenate([[0], np.cumsum(nch_b)])
    NCHF, NCHB = int(cb_f[-1]), int(cb_b[-1])
    # pos column base per slot (edge chunks only), f then b merged
    pb_f = np.concatenate([[0], np.cumsum(K_f)])
    pb_b = np.concatenate([[0], np.cumsum(K_b)]) + pb_f[-1]
    PC = int(pb_b[-1])

    groups = [list(range(g, min(g + group_tiles, T)))
              for g in range(0, T, group_tiles)]

    # core/slot -> global tile id
    cs_tile = np.full((n_cores, T), -1, dtype=np.int64)
    cs_tile[tile_core, tile_slot] = np.arange(n_tiles)

    iota3 = np.ascontiguousarray(
        np.broadcast_to(np.arange(P, dtype=np.float16)[None, :, None],
                        (P, P, kmax)))
    ident = np.eye(P, dtype=np.float16)
    wf = np.ascontiguousarray(W_fwd.astype(BF16))
    wb = np.ascontiguousarray(W_bwd.astype(BF16))
    bias_mat = np.ascontiguousarray(np.broadcast_to(
        (b_fwd + b_bwd).astype(np.float32)[None, :], (P, P)))

    in_maps = []
    for core in range(n_cores):
        tiles = cs_tile[core]                     # [T] global tile ids
        nodes = tiles[:, None] * P + np.arange(P)[None, :]  # [T, P]
        valid = nodes < N
        nidx = np.where(valid, nodes, 0)

        def build(keys, gidx, u, K, cb, pb, NCH):
            # keys = destination node per edge, gidx = source node
            m = tile_core[node_tile[keys]] == core
            kk, gg = keys[m], gidx[m]
            t_loc = tile_slot[node_tile[kk]]
            o = np.argsort(t_loc, kind="stable")
            t_loc, kk, gg = t_loc[o], kk[o], gg[o]
            starts = np.searchsorted(t_loc, np.arange(T))
            cnt = np.arange(len(kk)) - starts[t_loc]
            chunk = cb[t_loc] + 1 + cnt // P
            lane = cnt % P
            tbl = np.zeros((P, NCH, P), dtype=np.float16)
            tbl[lane, chunk, :] = u[gg]
            # self chunks: permuted u rows of the tile's own nodes
            selfrows = np.where(valid[:, :, None], u[nidx], 0)  # [T, P, P]
            tbl[:, cb[:-1], :] = selfrows.transpose(1, 0, 2)
            posm = np.zeros((P, int(K.sum())), dtype=np.float16)
            posm[lane, pb[t_loc] - pb[0] + cnt // P] = (kk % P)
            return tbl.reshape(P, NCH * P), posm

        tbl_f, pos_f = build(dst, src, u_f, K_f, cb_f, pb_f, NCHF)
        tbl_b, pos_b = build(src, dst, u_b, K_b, cb_b, pb_b, NCHB)
        pos = np.concatenate([pos_f, pos_b], axis=1)
        assert pos.shape[1] == PC

        dvf = np.zeros((P, T), dtype=np.float32)
        dvb = np.zeros((P, T), dtype=np.float32)
        dvf.T[valid] = dinv_f[nidx[valid]]
        dvb.T[valid] = dinv_b[nidx[valid]]

        in_maps.append({
            "tblf": tbl_f, "tblb": tbl_b, "pos": np.ascontiguousarray(pos),
            "dinvf": np.ascontiguousarray(dvf),
            "dinvb": np.ascontiguousarray(dvb),
            "wf": wf, "wb": wb, "bias": bias_mat,
            "iota": iota3, "ident": ident,
        })

    sch = Schedule()
    sch.T, sch.kmax, sch.groups = T, kmax, groups
    sch.K_f, sch.K_b = K_f, K_b
    sch.cb_f, sch.cb_b = cb_f, cb_b
    sch.pb_f, sch.pb_b = pb_f, pb_b
    sch.NCHF, sch.NCHB, sch.PC = NCHF, NCHB, PC
    sch.in_maps = in_maps
    sch.cs_tile = cs_tile
    sch.N, sch.n_cores = N, n_cores
    return sch


# ---------------------------------------------------------------------------
# device program
# ---------------------------------------------------------------------------

def build_program(sch, debug=False, dup=1):
    from contextlib import ExitStack
    import concourse.mybir as mybir
    import concourse.tile as tile
    from concourse import bacc

    f16 = mybir.dt.float16
    bf16 = mybir.dt.bfloat16
    f32 = mybir.dt.float32
    T, kmax = sch.T, sch.kmax
    K_f, K_b = sch.K_f, sch.K_b
    cb_f, cb_b = sch.cb_f, sch.cb_b
    pb_f, pb_b = sch.pb_f, sch.pb_b

    nc = bacc.Bacc("TRN2", target_bir_lowering=False, debug=debug,
                   num_devices=sch.n_cores, num_swdge_queues=1)

    tblf_d = nc.dram_tensor("tblf", [P, sch.NCHF * P], f16,
                            kind="ExternalInput").ap()
    tblb_d = nc.dram_tensor("tblb", [P, sch.NCHB * P], f16,
                            kind="ExternalInput").ap()
    pos_d = nc.dram_tensor("pos", [P, sch.PC], f16,
                           kind="ExternalInput").ap()
    dinvf_d = nc.dram_tensor("dinvf", [P, T], f32, kind="ExternalInput").ap()
    dinvb_d = nc.dram_tensor("dinvb", [P, T], f32, kind="ExternalInput").ap()
    wf_d = nc.dram_tensor("wf", [P, P], bf16, kind="ExternalInput").ap()
    wb_d = nc.dram_tensor("wb", [P, P], bf16, kind="ExternalInput").ap()
    bias_d = nc.dram_tensor("bias", [P, P], f32, kind="ExternalInput").ap()
    iota_d = nc.dram_tensor("iota", [P, P, kmax], f16,
                            kind="ExternalInput").ap()
    ident_d = nc.dram_tensor("ident", [P, P], f16,
                             kind="ExternalInput").ap()
    out_d = nc.dram_tensor("out", [P, T * P], bf16,
                           kind="ExternalOutput").ap()

    with tile.TileContext(nc) as tc, ExitStack() as ctx:
        const = ctx.enter_context(tc.tile_pool(name="const", bufs=1))

        def load_const(shape, dt, dram_ap, tag):
            t = const.tile(shape, dtype=dt, tag=tag)
            nc.sync.dma_start(out=t[:], in_=dram_ap)
            return t

        pos_sb = load_const([P, sch.PC], f16, pos_d, "pos")
        dinvf_sb = load_const([P, T], f32, dinvf_d, "dinvf")
        dinvb_sb = load_const([P, T], f32, dinvb_d, "dinvb")
        wf_sb = load_const([P, P], bf16, wf_d, "wf")
        wb_sb = load_const([P, P], bf16, wb_d, "wb")
        bias_sb = load_const([P, P], f32, bias_d, "bias")
        iota_sb = load_const([P, P, kmax], f16, iota_d, "iota")
        ident_sb = load_const([P, P], f16, ident_d, "ident")

        gpf = ctx.enter_context(tc.tile_pool(name="gf", bufs=3))
        gpb = ctx.enter_context(tc.tile_pool(name="gb", bufs=3))
        mpool = ctx.enter_context(tc.tile_pool(name="m", bufs=4))
        aggp = ctx.enter_context(tc.tile_pool(name="aggp", bufs=2,
                                              space="PSUM"))
        outp = ctx.enter_context(tc.tile_pool(name="outp", bufs=2,
                                              space="PSUM"))
        spool = ctx.enter_context(tc.tile_pool(name="sp", bufs=4))
        s1pool = ctx.enter_context(tc.tile_pool(name="s1p", bufs=3))
        s2pool = ctx.enter_context(tc.tile_pool(name="s2p", bufs=3))
        s3pool = ctx.enter_context(tc.tile_pool(name="s3p", bufs=3))
        ogrp = ctx.enter_context(tc.tile_pool(name="og", bufs=2))

        for _dup in range(dup):
            for g in sch.groups:
                g0, g1 = g[0], g[-1] + 1
                ncf = int(cb_f[g1] - cb_f[g0])
                ncb = int(cb_b[g1] - cb_b[g0])
                gtf = gpf.tile([P, ncf * P], dtype=f16, tag="gtf")
                nc.sync.dma_start(
                    out=gtf[:],
                    in_=tblf_d[:, int(cb_f[g0]) * P:int(cb_f[g1]) * P])
                gtb = gpb.tile([P, ncb * P], dtype=f16, tag="gtb")
                nc.sync.dma_start(
                    out=gtb[:],
                    in_=tblb_d[:, int(cb_b[g0]) * P:int(cb_b[g1]) * P])

                og_t = ogrp.tile([P, len(g), P], dtype=bf16, tag="og")

                for ti, t in enumerate(g):
                    def conv(gt, cb, pb, K, w_sb, tag):
                        kst = int(K[t])
                        base = int(cb[t] - cb[g0]) * P
                        agg = aggp.tile([P, P], dtype=f32, tag=f"agg{tag}")
                        nc.tensor.matmul(out=agg[:],
                                         lhsT=gt[:, base:base + P],
                                         rhs=ident_sb[:],
                                         start=True, stop=(kst == 0))
                        if kst > 0:
                            mt = mpool.tile([P, P, kst], dtype=f16,
                                            tag=f"m{tag}")
                            p0 = int(pb[t])
                            nc.vector.tensor_tensor(
                                out=mt[:],
                                in0=pos_sb[:, None, p0:p0 + kst]
                                .to_broadcast([P, P, kst]),
                                in1=iota_sb[:, :, 0:kst],
                                op=mybir.AluOpType.is_equal)
                            for c in range(kst):
                                nc.tensor.matmul(
                                    out=agg[:],
                                    lhsT=gt[:, base + (1 + c) * P:
                                            base + (2 + c) * P],
                                    rhs=mt[:, :, c],
                                    start=False, stop=(c == kst - 1))
                        a_sb = spool.tile([P, P], dtype=bf16, tag=f"a{tag}")
                        nc.scalar.copy(out=a_sb[:], in_=agg[:])
                        o_ps = outp.tile([P, P], dtype=f32, tag=f"o{tag}")
                        nc.tensor.matmul(out=o_ps[:], lhsT=a_sb[:],
                                         rhs=w_sb[:], start=True, stop=True)
                        return o_ps

                    outf = conv(gtf, cb_f, pb_f, K_f, wf_sb, "f")
                    outb = conv(gtb, cb_b, pb_b, K_b, wb_sb, "b")

                    s1 = s1pool.tile([P, P], dtype=f32, tag="s1")
                    nc.scalar.mul(out=s1[:], in_=outf[:],
                                  mul=dinvf_sb[:, t:t + 1])
                    s2 = s2pool.tile([P, P], dtype=f32, tag="s2")
                    nc.vector.scalar_tensor_tensor(
                        out=s2[:], in0=outb[:],
                        scalar=dinvb_sb[:, t:t + 1], in1=s1[:],
                        op0=mybir.AluOpType.mult,
                        op1=mybir.AluOpType.add)
                    s3 = s3pool.tile([P, P], dtype=f32, tag="s3")
                    nc.gpsimd.tensor_tensor(out=s3[:], in0=s2[:],
                                            in1=bias_sb[:],
                                            op=mybir.AluOpType.add)
                    nc.gpsimd.tensor_scalar(
                        out=og_t[:, ti, :], in0=s3[:],
                        scalar1=0.0, scalar2=None,
                        op0=mybir.AluOpType.max)

                nc.scalar.dma_start(out=out_d[:, g0 * P:g1 * P],
                                    in_=og_t[:])

    nc.compile()
    return nc


# ---------------------------------------------------------------------------
# entry point
# ---------------------------------------------------------------------------

_CACHE = {}


def run_sch(sch, trace=False, **kw):
    from concourse.bass_utils import run_bass_kernel_spmd
    key = ("prog", sch.T, sch.kmax, sch.NCHF, sch.NCHB, sch.PC)
    if key not in _CACHE:
        _CACHE.clear()
        _CACHE[key] = build_program(sch)
    nc = _CACHE[key]
    return run_bass_kernel_spmd(
        nc, sch.in_maps, core_ids=list(range(sch.n_cores)), trace=trace, **kw)


def assemble(sch, results):
    out = np.zeros((sch.N, P), dtype=np.float32)
    for core in range(sch.n_cores):
        o = np.asarray(results[core]["out"]).astype(np.float32)
        o = o.reshape(P, sch.T, P).transpose(1, 0, 2)
        tiles = sch.cs_tile[core]
        nodes = tiles[:, None] * P + np.arange(P)[None, :]
        valid = nodes < sch.N
        out[nodes[valid]] = o[valid]
    return out


def kernel(x, edge_index, W_fwd, b_fwd, W_bwd, b_bwd):
    x = np.asarray(x, dtype=np.float32)
    edge_index = np.asarray(edge_index, dtype=np.int32)
    sch = preprocess(
        x, edge_index,
        np.asarray(W_fwd, np.float32), np.asarray(b_fwd, np.float32),
        np.asarray(W_bwd, np.float32), np.asarray(b_bwd, np.float32))
    res = run_sch(sch)
    return assemble(sch, res.results)
